# revision 3
# baseline (speedup 1.0000x reference)
import os
import sys

sys.path.insert(0, "/opt/trn_rl_repo")

import numpy as np
from concourse import bacc, tile, mybir, bass_utils

F32 = mybir.dt.float32
F32R = mybir.dt.float32r
Act = mybir.ActivationFunctionType
Alu = mybir.AluOpType

B, S, D = 2, 2048, 1024
H, DK = 16, 64
HPG = 4              # heads per core (head-group)
CPG = HPG * DK       # 256 channels per core
NCORES = 8
NKC = D // 128       # 8 contraction chunks for projections
NJ = S // 128        # 16 key chunks
NW = S // 512        # 4 query windows

_CACHE = {}


def _build():
    if "nc" in _CACHE:
        return _CACHE["nc"]

    nc = bacc.Bacc("TRN2", debug=False, num_devices=1)

    xq_d = nc.dram_tensor("xq", [D, S], F32R, kind="ExternalInput")
    xk_d = nc.dram_tensor("xk", [D, S], F32R, kind="ExternalInput")
    xv_d = nc.dram_tensor("xv", [D, S], F32R, kind="ExternalInput")
    wq_d = nc.dram_tensor("wq", [D, CPG], F32R, kind="ExternalInput")
    wk_d = nc.dram_tensor("wk", [D, CPG], F32R, kind="ExternalInput")
    wv_d = nc.dram_tensor("wv", [D, CPG], F32R, kind="ExternalInput")
    wo_d = nc.dram_tensor("wo", [CPG, D], F32R, kind="ExternalInput")
    bq_d = nc.dram_tensor("bq", [128, 2], F32, kind="ExternalInput")
    bk_d = nc.dram_tensor("bk", [128, 2], F32, kind="ExternalInput")
    out_d = nc.dram_tensor("out", [S, D], F32, kind="ExternalOutput")

    with tile.TileContext(nc) as tc:
        with tc.tile_pool(name="wp", bufs=1) as wp, \
             tc.tile_pool(name="xp", bufs=3) as xp, \
             tc.tile_pool(name="bp", bufs=1) as bp, \
             tc.tile_pool(name="sp", bufs=2) as sp, \
             tc.tile_pool(name="ep", bufs=3) as ep, \
             tc.tile_pool(name="op", bufs=3) as op_, \
             tc.tile_pool(name="ps", bufs=1, space="PSUM") as ps:

            # ---- weights / biases to SBUF ----
            wq_sb = wp.tile([128, NKC, CPG], F32R, name="wq_sb")
            wk_sb = wp.tile([128, NKC, CPG], F32R, name="wk_sb")
            wv_sb = wp.tile([128, NKC, CPG], F32R, name="wv_sb")
            # wo laid out per-head on 64 partitions: [64, HPG, D]
            wo_sb = wp.tile([64, HPG, D], F32R, name="wo_sb")
            for kk in range(NKC):
                nc.sync.dma_start(out=wq_sb[:, kk, :], in_=wq_d[kk * 128:(kk + 1) * 128, :])
                nc.sync.dma_start(out=wk_sb[:, kk, :], in_=wk_d[kk * 128:(kk + 1) * 128, :])
                nc.sync.dma_start(out=wv_sb[:, kk, :], in_=wv_d[kk * 128:(kk + 1) * 128, :])
            for h in range(HPG):
                nc.sync.dma_start(out=wo_sb[:, h, :], in_=wo_d[h * 64:(h + 1) * 64, :])
            bq_sb = wp.tile([128, 2], F32, name="bq_sb")
            bk_sb = wp.tile([128, 2], F32, name="bk_sb")
            nc.sync.dma_start(out=bq_sb[:], in_=bq_d[:])
            nc.sync.dma_start(out=bk_sb[:], in_=bk_d[:])

            # constants
            vones32 = wp.tile([128, HPG], F32, name="vones32")
            nc.vector.memset(vones32[:], 1.0)
            ones32 = wp.tile([1, 64], F32, name="ones32")
            nc.vector.memset(ones32[:], 1.0)
            ones_r = wp.tile([1, 64], F32R, name="ones_r")
            nc.scalar.activation(ones_r[:], ones32[:], Act.Copy)

            # ---- persistent big SBUF tensors ----
            kt = [bp.tile([128, S], F32R, name=f"kt{m}") for m in range(2)]
            qt = [bp.tile([128, S], F32R, name=f"qt{m}") for m in range(2)]
            ctx = [bp.tile([64, S], F32R, name=f"ctx{h}") for h in range(HPG)]
            vaug = [bp.tile([128, HPG, DK + 1], F32R, name=f"va{j}") for j in range(NJ)]

            # ---- phase 1: K projection  KT[256, 2048] = Wk_slice @ x_k^T + b_k
            pk = [ps.tile([128, 512], F32, name=f"p{i}") for i in range(8)]
            for kk in range(NKC):
                xt = xp.tile([128, S], F32R, name="xt")
                nc.sync.dma_start(out=xt[:], in_=xk_d[kk * 128:(kk + 1) * 128, :])
                for m in range(2):
                    for n in range(4):
                        nc.tensor.matmul(
                            pk[4 * m + n][:],
                            wk_sb[:, kk, m * 128:(m + 1) * 128],
                            xt[:, n * 512:(n + 1) * 512],
                            start=(kk == 0), stop=(kk == NKC - 1))
            for m in range(2):
                for n in range(4):
                    nc.scalar.activation(
                        kt[m][:, n * 512:(n + 1) * 512], pk[4 * m + n][:],
                        Act.Identity, bias=bk_sb[:, m:m + 1])

            # ---- phase 2: V projection (s-major) + augmented ones column
            # one accumulation group per PSUM bank (a start=True matmul
            # clobbers the whole bank), so process 8 s-chunks at a time
            for half in range(2):
                pv = [ps.tile([128, 512], F32, name=f"p{i}") for i in range(8)]
                for kk in range(NKC):
                    xth = xp.tile([128, S // 2], F32R, name="xth")
                    nc.sync.dma_start(
                        out=xth[:],
                        in_=xv_d[kk * 128:(kk + 1) * 128,
                                 half * (S // 2):(half + 1) * (S // 2)])
                    for jj in range(8):
                        nc.tensor.matmul(
                            pv[jj][:, 0:256],
                            xth[:, jj * 128:(jj + 1) * 128],
                            wv_sb[:, kk, :],
                            start=(kk == 0), stop=(kk == NKC - 1))
                for jj in range(8):
                    j = half * 8 + jj
                    nc.scalar.activation(
                        vaug[j][:, :, 0:DK],
                        pv[jj][:, 0:256].rearrange("p (h d) -> p h d", h=HPG),
                        Act.Copy)
                    nc.scalar.activation(
                        vaug[j][:, :, DK:DK + 1].squeeze(2), vones32[:], Act.Copy)

            # ---- phase 3: Q projection with 1/8 scale folded in
            pq = [ps.tile([128, 512], F32, name=f"p{i}") for i in range(8)]
            for kk in range(NKC):
                xt = xp.tile([128, S], F32R, name="xt")
                nc.sync.dma_start(out=xt[:], in_=xq_d[kk * 128:(kk + 1) * 128, :])
                for m in range(2):
                    for n in range(4):
                        nc.tensor.matmul(
                            pq[4 * m + n][:],
                            wq_sb[:, kk, m * 128:(m + 1) * 128],
                            xt[:, n * 512:(n + 1) * 512],
                            start=(kk == 0), stop=(kk == NKC - 1))
            for m in range(2):
                for n in range(4):
                    nc.scalar.activation(
                        qt[m][:, n * 512:(n + 1) * 512], pq[4 * m + n][:],
                        Act.Identity, bias=bq_sb[:, m:m + 1], scale=0.125)

            # ---- phase 4: attention per (query window, head) + out-proj per window
            for w in range(NW):
                for h in range(HPG):
                    th = h // 2
                    po = (h % 2) * 64
                    pc = ps.tile([128, 512], F32, name="p3")
                    for j in range(NJ):
                        pst = ps.tile([128, 512], F32, name=f"p{j % 3}")
                        nc.tensor.matmul(
                            pst[:],
                            kt[th][po:po + 64, j * 128:(j + 1) * 128],
                            qt[th][po:po + 64, w * 512:(w + 1) * 512],
                            start=True, stop=True)
                        pe_t = ep.tile([128, 512], F32R, name="pexp")
                        nc.scalar.activation(pe_t[:], pst[:], Act.Exp)
                        nc.tensor.matmul(
                            pc[0:DK + 1, :], vaug[j][:, h, :], pe_t[:],
                            start=(j == 0), stop=(j == NJ - 1))
                    recip = sp.tile([1, 512], F32, name="recip")
                    nc.vector.reciprocal(recip[:], pc[DK:DK + 1, :])
                    recip_r = sp.tile([1, 512], F32R, name="recip_r")
                    nc.scalar.activation(recip_r[:], recip[:], Act.Copy)
                    prb = ps.tile([128, 512], F32, name="p4")
                    nc.tensor.matmul(prb[0:64, :], ones_r[:], recip_r[:],
                                     start=True, stop=True)
                    craw = sp.tile([64, 512], F32, name="craw")
                    nc.scalar.activation(craw[:], pc[0:DK, :], Act.Copy)
                    nc.vector.scalar_tensor_tensor(
                        ctx[h][:, w * 512:(w + 1) * 512],
                        craw[:], 1.0, prb[0:64, :],
                        op0=Alu.mult, op1=Alu.mult)

                # out-proj for this window: out[s, :] = sum_h ctx_h^T @ Wo_h
                for mi in range(4):
                    m = w * 4 + mi
                    for n in range(2):
                        pso = ps.tile([128, 512], F32, name=f"p{6 + n}")
                        for h in range(HPG):
                            nc.tensor.matmul(
                                pso[:],
                                ctx[h][:, m * 128:(m + 1) * 128],
                                wo_sb[:, h, n * 512:(n + 1) * 512],
                                start=(h == 0), stop=(h == HPG - 1))
                        osb = op_.tile([128, 512], F32, name="osb")
                        nc.vector.tensor_copy(osb[:], pso[:])
                        nc.sync.dma_start(
                            out=out_d[m * 128:(m + 1) * 128, n * 512:(n + 1) * 512],
                            in_=osb[:])

    nc.compile()
    _CACHE["nc"] = nc
    return nc


def kernel(**inputs):
    q = np.asarray(inputs["q"], np.float32)
    k = np.asarray(inputs["k"], np.float32)
    v = np.asarray(inputs["v"], np.float32)
    W_q = np.asarray(inputs["W_q"], np.float32)
    W_k = np.asarray(inputs["W_k"], np.float32)
    W_v = np.asarray(inputs["W_v"], np.float32)
    W_o = np.asarray(inputs["W_o"], np.float32)
    b_q = np.asarray(inputs["b_q"], np.float32)
    b_k = np.asarray(inputs["b_k"], np.float32)
    b_v = np.asarray(inputs["b_v"], np.float32)
    b_o = np.asarray(inputs["b_o"], np.float32)

    nc = _build()

    xqT = [np.ascontiguousarray(q[b].T) for b in range(B)]
    xkT = [np.ascontiguousarray(k[b].T) for b in range(B)]
    xvT = [np.ascontiguousarray(v[b].T) for b in range(B)]

    in_maps = []
    for c in range(NCORES):
        b, hg = c // 4, c % 4
        sl = slice(hg * CPG, (hg + 1) * CPG)
        in_maps.append({
            "xq": xqT[b],
            "xk": xkT[b],
            "xv": xvT[b],
            "wq": np.ascontiguousarray(W_q[sl, :].T),
            "wk": np.ascontiguousarray(W_k[sl, :].T),
            "wv": np.ascontiguousarray(W_v[sl, :].T),
            "wo": np.ascontiguousarray(W_o[:, sl].T),
            "bq": np.ascontiguousarray((b_q[sl] / 8.0).reshape(2, 128).T),
            "bk": np.ascontiguousarray(b_k[sl].reshape(2, 128).T),
        })

    trace = os.environ.get("BASS_KERNEL_TRACE") == "1"
    res = bass_utils.run_bass_kernel_spmd(nc, in_maps, list(range(NCORES)),
                                          trace=trace)
    kernel.last_exec_time_ns = getattr(res, "exec_time_ns", None)

    out = np.zeros((B, S, D), np.float32)
    for c in range(NCORES):
        out[c // 4] += res.results[c]["out"]
    out += (b_v @ W_o.T + b_o)[None, None, :]
    return out


# revision 5
# speedup vs baseline: 1.5522x; 1.5522x over previous
import os
import sys

sys.path.insert(0, "/opt/trn_rl_repo")

import ml_dtypes
import numpy as np
from concourse import bacc, tile, mybir, bass_utils

F32 = mybir.dt.float32
F32R = mybir.dt.float32r
BF16 = mybir.dt.bfloat16
Act = mybir.ActivationFunctionType
Alu = mybir.AluOpType

B, S, D = 2, 2048, 1024
H, DK = 16, 64
HPG = 4              # heads per core (head-group)
CPG = HPG * DK       # 256 channels per core
NCORES = 8
NKC = D // 128       # 8 contraction chunks for projections
NJ = S // 128        # 16 key chunks
NW = S // 512        # 4 query windows

_CACHE = {}


def _build():
    if "nc" in _CACHE:
        return _CACHE["nc"]

    nc = bacc.Bacc("TRN2", debug=False, num_devices=1)

    xq_d = nc.dram_tensor("xq", [D, S], BF16, kind="ExternalInput")
    xk_d = nc.dram_tensor("xk", [D, S], BF16, kind="ExternalInput")
    xv_d = nc.dram_tensor("xv", [D, S], BF16, kind="ExternalInput")
    wq_d = nc.dram_tensor("wq", [D, CPG], BF16, kind="ExternalInput")
    wk_d = nc.dram_tensor("wk", [D, CPG], BF16, kind="ExternalInput")
    wv_d = nc.dram_tensor("wv", [D, CPG], BF16, kind="ExternalInput")
    wo_d = nc.dram_tensor("wo", [CPG, D], F32R, kind="ExternalInput")
    bq_d = nc.dram_tensor("bq", [128, 2], F32, kind="ExternalInput")
    bk_d = nc.dram_tensor("bk", [128, 2], F32, kind="ExternalInput")
    out_d = nc.dram_tensor("out", [S, D], F32, kind="ExternalOutput")

    with tile.TileContext(nc) as tc:
        with tc.tile_pool(name="wp", bufs=1) as wp, \
             tc.tile_pool(name="xp", bufs=3) as xp, \
             tc.tile_pool(name="bp", bufs=1) as bp, \
             tc.tile_pool(name="sp", bufs=2) as sp, \
             tc.tile_pool(name="ep", bufs=5) as ep, \
             tc.tile_pool(name="op", bufs=3) as op_, \
             tc.tile_pool(name="ps", bufs=1, space="PSUM") as ps:

            # constants + exp table preload (before any other ACT work)
            vones32 = wp.tile([128, HPG], F32, name="vones32")
            nc.vector.memset(vones32[:], 1.0)
            dummy_e = wp.tile([1, HPG], F32, name="dummy_e")
            nc.scalar.activation(dummy_e[:], vones32[0:1, :], Act.Exp)
            ones32 = wp.tile([1, 64], F32, name="ones32")
            nc.vector.memset(ones32[:], 1.0)
            ones_r = wp.tile([1, 64], F32R, name="ones_r")
            nc.vector.tensor_copy(ones_r[:], ones32[:])

            # ---- weights / biases to SBUF ----
            wq_sb = wp.tile([128, NKC, CPG], BF16, name="wq_sb")
            wk_sb = wp.tile([128, NKC, CPG], BF16, name="wk_sb")
            wv_sb = wp.tile([128, NKC, CPG], BF16, name="wv_sb")
            wo_sb = wp.tile([64, HPG, D], F32R, name="wo_sb")
            for kk in range(NKC):
                nc.sync.dma_start(out=wk_sb[:, kk, :], in_=wk_d[kk * 128:(kk + 1) * 128, :])
                nc.sync.dma_start(out=wv_sb[:, kk, :], in_=wv_d[kk * 128:(kk + 1) * 128, :])
                nc.sync.dma_start(out=wq_sb[:, kk, :], in_=wq_d[kk * 128:(kk + 1) * 128, :])
            for h in range(HPG):
                nc.sync.dma_start(out=wo_sb[:, h, :], in_=wo_d[h * 64:(h + 1) * 64, :])
            bq_sb = wp.tile([128, 2], F32, name="bq_sb")
            bk_sb = wp.tile([128, 2], F32, name="bk_sb")
            nc.sync.dma_start(out=bq_sb[:], in_=bq_d[:])
            nc.sync.dma_start(out=bk_sb[:], in_=bk_d[:])

            # ---- persistent big SBUF tensors ----
            kt = [bp.tile([128, S], F32R, name=f"kt{m}") for m in range(2)]
            qt = [bp.tile([128, S], F32R, name=f"qt{m}") for m in range(2)]
            ctx = [bp.tile([64, S], F32R, name=f"ctx{h}") for h in range(HPG)]
            vaug = [bp.tile([128, HPG, DK + 1], F32R, name=f"va{j}") for j in range(NJ)]

            # PSUM: 4 wide tiles x 2 banks = all 8 banks
            def pt(name):
                return ps.tile([128, 1024], F32, name=name)

            PN = ["pa0", "pa1", "pb", "px"]

            # ---- phase 1: K projection  KT[256, 2048] = Wk_slice^T @ x_k
            pk = [pt(PN[i]) for i in range(4)]
            for kk in range(NKC):
                xt = xp.tile([128, S], BF16, name="xt")
                nc.sync.dma_start(out=xt[:], in_=xk_d[kk * 128:(kk + 1) * 128, :])
                for m in range(2):
                    for n in range(4):
                        g = 4 * m + n
                        nc.tensor.matmul(
                            pk[g // 2][:, (g % 2) * 512:(g % 2) * 512 + 512],
                            wk_sb[:, kk, m * 128:(m + 1) * 128],
                            xt[:, n * 512:(n + 1) * 512],
                            start=(kk == 0), stop=(kk == NKC - 1))
            for m in range(2):
                for t in range(2):
                    nc.scalar.activation(
                        kt[m][:, t * 1024:(t + 1) * 1024], pk[2 * m + t][:],
                        Act.Identity, bias=bk_sb[:, m:m + 1])

            # ---- phase 2: V projection (s-major) + augmented ones column
            for half in range(2):
                pv = [pt(PN[i]) for i in range(4)]
                for kk in range(NKC):
                    xth = xp.tile([128, S // 2], BF16, name="xth")
                    nc.sync.dma_start(
                        out=xth[:],
                        in_=xv_d[kk * 128:(kk + 1) * 128,
                                 half * (S // 2):(half + 1) * (S // 2)])
                    for jj in range(8):
                        nc.tensor.matmul(
                            pv[jj // 2][:, (jj % 2) * 512:(jj % 2) * 512 + 256],
                            xth[:, jj * 128:(jj + 1) * 128],
                            wv_sb[:, kk, :],
                            start=(kk == 0), stop=(kk == NKC - 1))
                for jj in range(8):
                    j = half * 8 + jj
                    nc.scalar.activation(
                        vaug[j][:, :, 0:DK],
                        pv[jj // 2][:, (jj % 2) * 512:(jj % 2) * 512 + 256]
                        .rearrange("p (h d) -> p h d", h=HPG),
                        Act.Copy)
                    nc.scalar.activation(
                        vaug[j][:, :, DK:DK + 1].squeeze(2), vones32[:], Act.Copy)

            # ---- phase 3: Q projection with 1/8 scale folded in (host side)
            pq = [pt(PN[i]) for i in range(4)]
            for kk in range(NKC):
                xt = xp.tile([128, S], BF16, name="xt")
                nc.sync.dma_start(out=xt[:], in_=xq_d[kk * 128:(kk + 1) * 128, :])
                for m in range(2):
                    for n in range(4):
                        g = 4 * m + n
                        nc.tensor.matmul(
                            pq[g // 2][:, (g % 2) * 512:(g % 2) * 512 + 512],
                            wq_sb[:, kk, m * 128:(m + 1) * 128],
                            xt[:, n * 512:(n + 1) * 512],
                            start=(kk == 0), stop=(kk == NKC - 1))
            for m in range(2):
                for t in range(2):
                    nc.scalar.activation(
                        qt[m][:, t * 1024:(t + 1) * 1024], pq[2 * m + t][:],
                        Act.Identity, bias=bq_sb[:, m:m + 1])

            # ---- phase 4: pair-packed attention, deferred normalize ----
            pending = None   # (w, p, pc2, rcp_r) awaiting prb/prs/stt emission
            po_ready = None  # window index whose out-proj is ready to emit

            def finish_normalize(st):
                pw, pp, pc2, rcp_r = st
                prb = pt("px")
                nc.tensor.matmul(prb[0:64, 0:512], ones_r[:], rcp_r[:, 0:512],
                                 start=True, stop=True)
                nc.tensor.matmul(prb[0:64, 512:1024], ones_r[:], rcp_r[:, 512:1024],
                                 start=True, stop=True)
                prs = sp.tile([64, 1024], F32R, name="prs")
                nc.vector.tensor_copy(prs[:], prb[0:64, :])
                for hh in range(2):
                    nc.vector.scalar_tensor_tensor(
                        ctx[2 * pp + hh][:, pw * 512:(pw + 1) * 512],
                        pc2[0:DK, hh * 512:(hh + 1) * 512], 1.0,
                        prs[:, hh * 512:(hh + 1) * 512],
                        op0=Alu.mult, op1=Alu.mult)

            def emit_outproj_group(pw, mi):
                m = pw * 4 + mi
                po = pt("px")
                for n in range(2):
                    for h in range(HPG):
                        nc.tensor.matmul(
                            po[:, n * 512:(n + 1) * 512],
                            ctx[h][:, m * 128:(m + 1) * 128],
                            wo_sb[:, h, n * 512:(n + 1) * 512],
                            start=(h == 0), stop=(h == HPG - 1))
                osb = op_.tile([128, 1024], F32, name="osb")
                nc.vector.tensor_copy(osb[:], po[:])
                nc.sync.dma_start(out=out_d[m * 128:(m + 1) * 128, :], in_=osb[:])

            for gidx in range(2 * NW):
                w, p = gidx >> 1, gidx & 1
                pc2 = pt("pb")
                for j in range(NJ):
                    pst = pt(f"pa{j % 2}")
                    nc.tensor.matmul(
                        pst[:, 0:512],
                        kt[p][0:64, j * 128:(j + 1) * 128],
                        qt[p][0:64, w * 512:(w + 1) * 512],
                        start=True, stop=True)
                    nc.tensor.matmul(
                        pst[:, 512:1024],
                        kt[p][64:128, j * 128:(j + 1) * 128],
                        qt[p][64:128, w * 512:(w + 1) * 512],
                        start=True, stop=True)
                    pe_t = ep.tile([128, 1024], F32R, name="pexp")
                    nc.scalar.activation(pe_t[:], pst[:], Act.Exp)
                    nc.tensor.matmul(
                        pc2[0:DK + 1, 0:512], vaug[j][:, 2 * p, :],
                        pe_t[:, 0:512],
                        start=(j == 0), stop=(j == NJ - 1))
                    nc.tensor.matmul(
                        pc2[0:DK + 1, 512:1024], vaug[j][:, 2 * p + 1, :],
                        pe_t[:, 512:1024],
                        start=(j == 0), stop=(j == NJ - 1))
                    if j == 1 and pending is not None:
                        finish_normalize(pending)
                        pending = None
                    if po_ready is not None and j in (3, 6, 9, 12):
                        emit_outproj_group(po_ready, (j - 3) // 3)
                        if j == 12:
                            po_ready = None
                # start normalize for this pair: denom -> recip -> f32r
                den = sp.tile([1, 1024], F32, name="den")
                nc.vector.tensor_copy(den[:], pc2[DK:DK + 1, :])
                rcp = sp.tile([1, 1024], F32, name="rcp")
                nc.vector.reciprocal_approx_fast(out=rcp[:], in_=den[:])
                rcp_r = sp.tile([1, 1024], F32R, name="rcp_r")
                nc.vector.tensor_copy(rcp_r[:], rcp[:])
                pending = (w, p, pc2, rcp_r)
                if p == 1:
                    po_ready = w

            # tail: last pair normalize + last window out-proj
            finish_normalize(pending)
            for mi in range(4):
                emit_outproj_group(NW - 1, mi)

    nc.compile()
    _CACHE["nc"] = nc
    return nc


def kernel(**inputs):
    q = np.asarray(inputs["q"], np.float32)
    k = np.asarray(inputs["k"], np.float32)
    v = np.asarray(inputs["v"], np.float32)
    W_q = np.asarray(inputs["W_q"], np.float32)
    W_k = np.asarray(inputs["W_k"], np.float32)
    W_v = np.asarray(inputs["W_v"], np.float32)
    W_o = np.asarray(inputs["W_o"], np.float32)
    b_q = np.asarray(inputs["b_q"], np.float32)
    b_k = np.asarray(inputs["b_k"], np.float32)
    b_v = np.asarray(inputs["b_v"], np.float32)
    b_o = np.asarray(inputs["b_o"], np.float32)

    nc = _build()

    bf = ml_dtypes.bfloat16
    xqT = [np.ascontiguousarray(q[b].T).astype(bf) for b in range(B)]
    xkT = [np.ascontiguousarray(k[b].T).astype(bf) for b in range(B)]
    xvT = [np.ascontiguousarray(v[b].T).astype(bf) for b in range(B)]

    in_maps = []
    for c in range(NCORES):
        b, hg = c // 4, c % 4
        sl = slice(hg * CPG, (hg + 1) * CPG)
        in_maps.append({
            "xq": xqT[b],
            "xk": xkT[b],
            "xv": xvT[b],
            "wq": np.ascontiguousarray(W_q[sl, :].T / 8.0).astype(bf),
            "wk": np.ascontiguousarray(W_k[sl, :].T).astype(bf),
            "wv": np.ascontiguousarray(W_v[sl, :].T).astype(bf),
            "wo": np.ascontiguousarray(W_o[:, sl].T),
            "bq": np.ascontiguousarray((b_q[sl] / 8.0).reshape(2, 128).T),
            "bk": np.ascontiguousarray(b_k[sl].reshape(2, 128).T),
        })

    trace = os.environ.get("BASS_KERNEL_TRACE") == "1"
    res = bass_utils.run_bass_kernel_spmd(nc, in_maps, list(range(NCORES)),
                                          trace=trace)
    kernel.last_exec_time_ns = getattr(res, "exec_time_ns", None)
    kernel.last_res = res

    out = np.zeros((B, S, D), np.float32)
    for c in range(NCORES):
        out[c // 4] += res.results[c]["out"]
    out += (b_v @ W_o.T + b_o)[None, None, :]
    return out


# revision 7
# speedup vs baseline: 1.6143x; 1.0400x over previous
import os
import sys

sys.path.insert(0, "/opt/trn_rl_repo")

import ml_dtypes
import numpy as np
from concourse import bacc, tile, mybir, bass_utils

F32 = mybir.dt.float32
F32R = mybir.dt.float32r
BF16 = mybir.dt.bfloat16
Act = mybir.ActivationFunctionType
Alu = mybir.AluOpType

B, S, D = 2, 2048, 1024
H, DK = 16, 64
HPG = 4              # heads per core (head-group)
CPG = HPG * DK       # 256 channels per core
NCORES = 8
NKC = D // 128       # 8 contraction chunks for projections
NJ = S // 128        # 16 key chunks
NW = S // 512        # 4 query windows

_CACHE = {}


def _build():
    if "nc" in _CACHE:
        return _CACHE["nc"]

    nc = bacc.Bacc("TRN2", debug=False, num_devices=1)

    xq_d = nc.dram_tensor("xq", [D, S], BF16, kind="ExternalInput")
    xk_d = nc.dram_tensor("xk", [D, S], BF16, kind="ExternalInput")
    xv_d = nc.dram_tensor("xv", [D, S], BF16, kind="ExternalInput")
    wq_d = nc.dram_tensor("wq", [D, CPG], BF16, kind="ExternalInput")
    wk_d = nc.dram_tensor("wk", [D, CPG], BF16, kind="ExternalInput")
    wv_d = nc.dram_tensor("wv", [D, CPG], BF16, kind="ExternalInput")
    wo_d = nc.dram_tensor("wo", [CPG, D], F32R, kind="ExternalInput")
    bq_d = nc.dram_tensor("bq", [128, 2], F32, kind="ExternalInput")
    bk_d = nc.dram_tensor("bk", [128, 2], F32, kind="ExternalInput")
    out_d = nc.dram_tensor("out", [S, D], F32, kind="ExternalOutput")

    with tile.TileContext(nc) as tc:
        with tc.tile_pool(name="wp", bufs=1) as wp, \
             tc.tile_pool(name="xp", bufs=3) as xp, \
             tc.tile_pool(name="bp", bufs=1) as bp, \
             tc.tile_pool(name="sp", bufs=2) as sp, \
             tc.tile_pool(name="ep", bufs=5) as ep, \
             tc.tile_pool(name="op", bufs=3) as op_, \
             tc.tile_pool(name="ps", bufs=1, space="PSUM") as ps:

            # constants + exp table preload (before any other ACT work)
            vones32 = wp.tile([128, HPG], F32, name="vones32")
            nc.vector.memset(vones32[:], 1.0)
            dummy_e = wp.tile([1, HPG], F32, name="dummy_e")
            nc.scalar.activation(dummy_e[:], vones32[0:1, :], Act.Exp)
            ones32 = wp.tile([1, 64], F32, name="ones32")
            nc.vector.memset(ones32[:], 1.0)
            ones_r = wp.tile([1, 64], F32R, name="ones_r")
            nc.vector.tensor_copy(ones_r[:], ones32[:])

            # ---- weights / biases to SBUF ----
            wq_sb = wp.tile([128, NKC, CPG], BF16, name="wq_sb")
            wk_sb = wp.tile([128, NKC, CPG], BF16, name="wk_sb")
            wv_sb = wp.tile([128, NKC, CPG], BF16, name="wv_sb")
            wo_sb = wp.tile([64, HPG, D], F32R, name="wo_sb")
            for kk in range(NKC):
                nc.sync.dma_start(out=wk_sb[:, kk, :], in_=wk_d[kk * 128:(kk + 1) * 128, :])
                nc.sync.dma_start(out=wv_sb[:, kk, :], in_=wv_d[kk * 128:(kk + 1) * 128, :])
                nc.sync.dma_start(out=wq_sb[:, kk, :], in_=wq_d[kk * 128:(kk + 1) * 128, :])
            for h in range(HPG):
                nc.sync.dma_start(out=wo_sb[:, h, :], in_=wo_d[h * 64:(h + 1) * 64, :])
            bq_sb = wp.tile([128, 2], F32, name="bq_sb")
            bk_sb = wp.tile([128, 2], F32, name="bk_sb")
            nc.sync.dma_start(out=bq_sb[:], in_=bq_d[:])
            nc.sync.dma_start(out=bk_sb[:], in_=bk_d[:])

            # ---- persistent big SBUF tensors ----
            kt = [bp.tile([128, S], BF16, name=f"kt{m}") for m in range(2)]
            qt = [bp.tile([128, S], BF16, name=f"qt{m}") for m in range(2)]
            ctx = [bp.tile([64, S], F32R, name=f"ctx{h}") for h in range(HPG)]
            vaug = [bp.tile([128, HPG, DK + 1], BF16, name=f"va{j}") for j in range(NJ)]

            # PSUM: 4 wide tiles x 2 banks = all 8 banks
            def pt(name):
                return ps.tile([128, 1024], F32, name=name)

            PN = ["pa0", "pa1", "pb", "px"]

            # ---- phase 1: K projection  KT[256, 2048] = Wk_slice^T @ x_k
            pk = [pt(PN[i]) for i in range(4)]
            for kk in range(NKC):
                xt = xp.tile([128, S], BF16, name="xt")
                nc.sync.dma_start(out=xt[:], in_=xk_d[kk * 128:(kk + 1) * 128, :])
                for m in range(2):
                    for n in range(4):
                        g = 4 * m + n
                        nc.tensor.matmul(
                            pk[g // 2][:, (g % 2) * 512:(g % 2) * 512 + 512],
                            wk_sb[:, kk, m * 128:(m + 1) * 128],
                            xt[:, n * 512:(n + 1) * 512],
                            start=(kk == 0), stop=(kk == NKC - 1))
            for m in range(2):
                for t in range(2):
                    nc.scalar.activation(
                        kt[m][:, t * 1024:(t + 1) * 1024], pk[2 * m + t][:],
                        Act.Identity, bias=bk_sb[:, m:m + 1])

            # ---- phase 2: V projection (s-major) + augmented ones column
            for half in range(2):
                pv = [pt(PN[i]) for i in range(4)]
                for kk in range(NKC):
                    xth = xp.tile([128, S // 2], BF16, name="xth")
                    nc.sync.dma_start(
                        out=xth[:],
                        in_=xv_d[kk * 128:(kk + 1) * 128,
                                 half * (S // 2):(half + 1) * (S // 2)])
                    for jj in range(8):
                        nc.tensor.matmul(
                            pv[jj // 2][:, (jj % 2) * 512:(jj % 2) * 512 + 256],
                            xth[:, jj * 128:(jj + 1) * 128],
                            wv_sb[:, kk, :],
                            start=(kk == 0), stop=(kk == NKC - 1))
                for jj in range(8):
                    j = half * 8 + jj
                    nc.scalar.activation(
                        vaug[j][:, :, 0:DK],
                        pv[jj // 2][:, (jj % 2) * 512:(jj % 2) * 512 + 256]
                        .rearrange("p (h d) -> p h d", h=HPG),
                        Act.Copy)
                    nc.scalar.activation(
                        vaug[j][:, :, DK:DK + 1].squeeze(2), vones32[:], Act.Copy)

            # ---- phase 3: Q projection with 1/8 scale folded in (host side)
            pq = [pt(PN[i]) for i in range(4)]
            for kk in range(NKC):
                xt = xp.tile([128, S], BF16, name="xt")
                nc.sync.dma_start(out=xt[:], in_=xq_d[kk * 128:(kk + 1) * 128, :])
                for m in range(2):
                    for n in range(4):
                        g = 4 * m + n
                        nc.tensor.matmul(
                            pq[g // 2][:, (g % 2) * 512:(g % 2) * 512 + 512],
                            wq_sb[:, kk, m * 128:(m + 1) * 128],
                            xt[:, n * 512:(n + 1) * 512],
                            start=(kk == 0), stop=(kk == NKC - 1))
            for m in range(2):
                for t in range(2):
                    nc.scalar.activation(
                        qt[m][:, t * 1024:(t + 1) * 1024], pq[2 * m + t][:],
                        Act.Identity, bias=bq_sb[:, m:m + 1])

            # ---- phase 4: pair-packed attention, deferred normalize ----
            pending = None   # (w, p, pc2, rcp_r) awaiting prb/prs/stt emission
            po_ready = None  # window index whose out-proj is ready to emit

            def finish_normalize(st):
                pw, pp, pc2, rcp_r = st
                prb = pt("px")
                nc.tensor.matmul(prb[0:64, 0:512], ones_r[:], rcp_r[:, 0:512],
                                 start=True, stop=True)
                nc.tensor.matmul(prb[0:64, 512:1024], ones_r[:], rcp_r[:, 512:1024],
                                 start=True, stop=True)
                prs = sp.tile([64, 1024], F32R, name="prs")
                nc.vector.tensor_copy(prs[:], prb[0:64, :])
                for hh in range(2):
                    nc.vector.scalar_tensor_tensor(
                        ctx[2 * pp + hh][:, pw * 512:(pw + 1) * 512],
                        pc2[0:DK, hh * 512:(hh + 1) * 512], 1.0,
                        prs[:, hh * 512:(hh + 1) * 512],
                        op0=Alu.mult, op1=Alu.mult)

            def emit_outproj_group(pw, mi):
                m = pw * 4 + mi
                po = pt("px")
                for n in range(2):
                    for h in range(HPG):
                        nc.tensor.matmul(
                            po[:, n * 512:(n + 1) * 512],
                            ctx[h][:, m * 128:(m + 1) * 128],
                            wo_sb[:, h, n * 512:(n + 1) * 512],
                            start=(h == 0), stop=(h == HPG - 1))
                osb = op_.tile([128, 1024], F32, name="osb")
                nc.vector.tensor_copy(osb[:], po[:])
                nc.sync.dma_start(out=out_d[m * 128:(m + 1) * 128, :], in_=osb[:])

            for gidx in range(2 * NW):
                w, p = gidx >> 1, gidx & 1
                pc2 = pt("pb")
                for j in range(NJ):
                    pst = pt(f"pa{j % 2}")
                    nc.tensor.matmul(
                        pst[:, 0:512],
                        kt[p][0:64, j * 128:(j + 1) * 128],
                        qt[p][0:64, w * 512:(w + 1) * 512],
                        start=True, stop=True)
                    nc.tensor.matmul(
                        pst[:, 512:1024],
                        kt[p][64:128, j * 128:(j + 1) * 128],
                        qt[p][64:128, w * 512:(w + 1) * 512],
                        start=True, stop=True)
                    pe_t = ep.tile([128, 1024], BF16, name="pexp")
                    nc.scalar.activation(pe_t[:], pst[:], Act.Exp)
                    nc.tensor.matmul(
                        pc2[0:DK + 1, 0:512], vaug[j][:, 2 * p, :],
                        pe_t[:, 0:512],
                        start=(j == 0), stop=(j == NJ - 1))
                    nc.tensor.matmul(
                        pc2[0:DK + 1, 512:1024], vaug[j][:, 2 * p + 1, :],
                        pe_t[:, 512:1024],
                        start=(j == 0), stop=(j == NJ - 1))
                    if j == 1 and pending is not None:
                        finish_normalize(pending)
                        pending = None
                    if po_ready is not None and j in (3, 6, 9, 12):
                        emit_outproj_group(po_ready, (j - 3) // 3)
                        if j == 12:
                            po_ready = None
                # start normalize for this pair: denom -> recip -> f32r
                den = sp.tile([1, 1024], F32, name="den")
                nc.vector.tensor_copy(den[:], pc2[DK:DK + 1, :])
                rcp = sp.tile([1, 1024], F32, name="rcp")
                nc.vector.reciprocal_approx_fast(out=rcp[:], in_=den[:])
                rcp_r = sp.tile([1, 1024], F32R, name="rcp_r")
                nc.vector.tensor_copy(rcp_r[:], rcp[:])
                pending = (w, p, pc2, rcp_r)
                if p == 1:
                    po_ready = w

            # tail: last pair normalize + last window out-proj
            finish_normalize(pending)
            for mi in range(4):
                emit_outproj_group(NW - 1, mi)

    nc.compile()
    _CACHE["nc"] = nc
    return nc


def kernel(**inputs):
    q = np.asarray(inputs["q"], np.float32)
    k = np.asarray(inputs["k"], np.float32)
    v = np.asarray(inputs["v"], np.float32)
    W_q = np.asarray(inputs["W_q"], np.float32)
    W_k = np.asarray(inputs["W_k"], np.float32)
    W_v = np.asarray(inputs["W_v"], np.float32)
    W_o = np.asarray(inputs["W_o"], np.float32)
    b_q = np.asarray(inputs["b_q"], np.float32)
    b_k = np.asarray(inputs["b_k"], np.float32)
    b_v = np.asarray(inputs["b_v"], np.float32)
    b_o = np.asarray(inputs["b_o"], np.float32)

    nc = _build()

    bf = ml_dtypes.bfloat16
    xqT = [np.ascontiguousarray(q[b].T).astype(bf) for b in range(B)]
    xkT = [np.ascontiguousarray(k[b].T).astype(bf) for b in range(B)]
    xvT = [np.ascontiguousarray(v[b].T).astype(bf) for b in range(B)]

    in_maps = []
    for c in range(NCORES):
        b, hg = c // 4, c % 4
        sl = slice(hg * CPG, (hg + 1) * CPG)
        in_maps.append({
            "xq": xqT[b],
            "xk": xkT[b],
            "xv": xvT[b],
            "wq": np.ascontiguousarray(W_q[sl, :].T / 8.0).astype(bf),
            "wk": np.ascontiguousarray(W_k[sl, :].T).astype(bf),
            "wv": np.ascontiguousarray(W_v[sl, :].T).astype(bf),
            "wo": np.ascontiguousarray(W_o[:, sl].T),
            "bq": np.ascontiguousarray((b_q[sl] / 8.0).reshape(2, 128).T),
            "bk": np.ascontiguousarray(b_k[sl].reshape(2, 128).T),
        })

    trace = os.environ.get("BASS_KERNEL_TRACE") == "1"
    res = bass_utils.run_bass_kernel_spmd(nc, in_maps, list(range(NCORES)),
                                          trace=trace)
    kernel.last_exec_time_ns = getattr(res, "exec_time_ns", None)
    kernel.last_res = res

    out = np.zeros((B, S, D), np.float32)
    for c in range(NCORES):
        out[c // 4] += res.results[c]["out"]
    out += (b_v @ W_o.T + b_o)[None, None, :]
    return out
